# revision 3
# baseline (speedup 1.0000x reference)
"""Causal multi-head attention block (qkv proj + attention + out proj) on 8 TRN2 cores.

Problem: x[4,2048,1024] @ Wqkv[1024,3072] -> 16-head causal attention -> @ Wout.

Sharding: batch(4) x head-group(2) -> 8 cores. Core c handles batch c//2 and
heads (c%2)*8..(c%2)*8+8. Each core computes its 8 heads' attention and a
partial out-projection [2048,1024]; host sums the two head-group partials per
batch and adds bout.

Device kernel (per core, all matmuls in float32r: full PE rate, ~1e-4 rounding):
  phase A/B per 512-token tile tt:
    QT/KT = (Wqkv cols)^T-layout proj   [outcol, t]  (weights stationary)
    V     = natural-layout proj          [t, outcol]  (x stationary)
  attention per (qt, head): S^T[k,q] tiles via KT x QT; exp on ACT (the 1/8
  scale is folded into Wq on host); causal diagonal masked by 0/1 mask mult;
  ctx'^T = [V|1]^T @ expS^T accumulated in PSUM -> row 64 is the softmax
  denominator; denominator broadcast via a K=1 PE matmul with a ones vector;
  reciprocal_approx_fast; normalize -> cn tiles [feat, q].
  out proj per qt: Y[q, out] += cn^T chunks @ Wout chunks.
"""
import numpy as np

B, T, C = 4, 2048, 1024
H, HD = 16, 64
NCORES = 8


def _build_program():
    import concourse.bacc as bacc
    import concourse.tile as tile
    from concourse import mybir

    dtf = mybir.dt.float32
    dtr = mybir.dt.float32r
    dtb = mybir.dt.bfloat16
    EXP = mybir.ActivationFunctionType.Exp
    CPY = mybir.ActivationFunctionType.Copy
    MULT = mybir.AluOpType.mult

    nc = bacc.Bacc('TRN2', target_bir_lowering=False, debug=False)
    xt_d = nc.dram_tensor("xt", [1024, 2048], dtr, kind="ExternalInput").ap()
    wqk_d = nc.dram_tensor("wqk", [1024, 1024], dtr, kind="ExternalInput").ap()
    wv_d = nc.dram_tensor("wv", [1024, 512], dtr, kind="ExternalInput").ap()
    wout_d = nc.dram_tensor("wout", [512, 1024], dtr, kind="ExternalInput").ap()
    mask_d = nc.dram_tensor("mask", [4, 128, 512], dtb, kind="ExternalInput").ap()
    bqk_d = nc.dram_tensor("bqk", [1024], dtf, kind="ExternalInput").ap()
    bv_d = nc.dram_tensor("bv", [512], dtf, kind="ExternalInput").ap()
    y_d = nc.dram_tensor("y", [2048, 1024], dtf, kind="ExternalOutput").ap()

    with tile.TileContext(nc) as tc:
        with tc.tile_pool(name="const", bufs=1) as const, \
             tc.tile_pool(name="kt_p", bufs=1) as kt_p, \
             tc.tile_pool(name="v_p", bufs=1) as v_p, \
             tc.tile_pool(name="xt_p", bufs=8) as xt_p, \
             tc.tile_pool(name="qt_p", bufs=6) as qt_p, \
             tc.tile_pool(name="cn_p", bufs=6) as cn_p, \
             tc.tile_pool(name="exp_p", bufs=4) as exp_p, \
             tc.tile_pool(name="y_p", bufs=2) as y_p, \
             tc.tile_pool(name="r_p", bufs=2) as r_p, \
             tc.tile_pool(name="rcp_p", bufs=2) as rcp_p, \
             tc.tile_pool(name="ps", bufs=6, space="PSUM") as ps, \
             tc.tile_pool(name="ps_ctx", bufs=2, space="PSUM") as ps_ctx:

            # ---- constants / weights ----
            wqk_sb = const.tile([128, 8, 8, 128], dtr)   # [p, kc, oc, c]
            wv_sb = const.tile([128, 8, 512], dtr)       # [p, kc, n]
            wout_sb = const.tile([128, 4, 2, 512], dtr)  # [p, hp, oc, c]
            masks = const.tile([128, 4, 512], dtb)
            bqk_sb = const.tile([128, 8], dtf)
            bv_sb = const.tile([128, 4], dtf)
            ones_t = const.tile([1, 128], dtr)
            ones_f32 = const.tile([1, 128], dtf)
            vcol_f32 = const.tile([128, 16, 8, 1], dtf)
            kt_store = const.tile([128, 4, 4, 512], dtr)  # [p, j, tt, t]
            v_all = v_p.tile([128, 16, 8, 65], dtr)       # [p, kt, h, d|1]

            for kc in range(8):
                nc.sync.dma_start(wqk_sb[:, kc, :, :],
                                  wqk_d[kc * 128:(kc + 1) * 128, :]
                                  .rearrange("p (oc c) -> p oc c", c=128))
            nc.sync.dma_start(wv_sb[:],
                              wv_d.rearrange("(kc p) n -> p kc n", p=128))
            nc.sync.dma_start(wout_sb[:],
                              wout_d.rearrange("(hp p) (oc c) -> p hp oc c",
                                               p=128, c=512))
            nc.sync.dma_start(masks[:], mask_d.rearrange("n p f -> p n f"))
            nc.sync.dma_start(bqk_sb[:], bqk_d.rearrange("(oc p) -> p oc", p=128))
            nc.sync.dma_start(bv_sb[:], bv_d.rearrange("(hp p) -> p hp", p=128))
            nc.vector.memset(ones_f32[:], 1.0)
            nc.vector.memset(vcol_f32[:], 1.0)
            nc.vector.tensor_copy(ones_t[:], ones_f32[:])
            nc.vector.tensor_copy(v_all[:, :, :, 64:65], vcol_f32[:])

            for tt in range(4):
                # ---- load x^T chunks for this 512-token tile ----
                xts = []
                for kc in range(8):
                    xt_t = xt_p.tile([128, 512], dtr, tag="xt")
                    nc.sync.dma_start(
                        xt_t[:], xt_d[kc * 128:(kc + 1) * 128,
                                      tt * 512:(tt + 1) * 512])
                    xts.append(xt_t)

                # ---- sweep 1: Q projection (transposed layout) ----
                qts = []
                for j in range(4):
                    psq = ps.tile([128, 512], dtf, tag="ps")
                    for kc in range(8):
                        nc.tensor.matmul(psq[:], wqk_sb[:, kc, j, :], xts[kc][:],
                                         start=(kc == 0), stop=(kc == 7))
                    qt_t = qt_p.tile([128, 512], dtr, tag="qt")
                    nc.vector.tensor_scalar_add(qt_t[:], psq[:], bqk_sb[:, j:j + 1])
                    qts.append(qt_t)

                # ---- sweep 2: K projection (transposed layout) ----
                for j in range(4):
                    psk = ps.tile([128, 512], dtf, tag="ps")
                    for kc in range(8):
                        nc.tensor.matmul(psk[:], wqk_sb[:, kc, 4 + j, :], xts[kc][:],
                                         start=(kc == 0), stop=(kc == 7))
                    nc.vector.tensor_scalar_add(kt_store[:, j, tt, :], psk[:],
                                                bqk_sb[:, 4 + j:5 + j])

                # ---- sweep 3: V projection (natural layout) ----
                for sub in range(4):
                    vt = tt * 4 + sub
                    psv = ps.tile([128, 512], dtf, tag="ps")
                    for kc in range(8):
                        nc.tensor.matmul(psv[:],
                                         xts[kc][:, sub * 128:(sub + 1) * 128],
                                         wv_sb[:, kc, :],
                                         start=(kc == 0), stop=(kc == 7))
                    nc.vector.tensor_copy(
                        v_all[:, vt, :, 0:64],
                        psv[:].rearrange("p (h d) -> p h d", h=8))

                # ---- attention for q-tile qt = tt ----
                qt = tt
                n_kt = 4 * qt + 4
                cns = []
                for hp in range(4):
                    ctx_e = ps_ctx.tile([65, 512], dtf, tag="ctx")
                    ctx_o = ps_ctx.tile([65, 512], dtf, tag="ctx")
                    for kt in range(n_kt):
                        ktt, kj = kt // 4, kt % 4
                        exps = []
                        pss = []
                        for hb in range(2):
                            h = 2 * hp + hb
                            pb = hb * 64
                            s_ps = ps.tile([128, 512], dtf, tag="ps")
                            nc.tensor.matmul(
                                s_ps[:],
                                kt_store[pb:pb + 64, hp, ktt,
                                         kj * 128:(kj + 1) * 128],
                                qts[hp][pb:pb + 64, :],
                                start=True, stop=True)
                            pss.append(s_ps)
                        for hb in range(2):
                            expS = exp_p.tile([128, 512], dtr, tag="exp")
                            nc.scalar.activation(expS[:], pss[hb][:], EXP)
                            if kt >= 4 * qt:
                                nc.vector.tensor_tensor(
                                    expS[:], expS[:], masks[:, kt - 4 * qt, :],
                                    MULT)
                            exps.append(expS)
                        for hb, ctx_ps in ((0, ctx_e), (1, ctx_o)):
                            h = 2 * hp + hb
                            nc.tensor.matmul(ctx_ps[:], v_all[:, kt, h, :],
                                             exps[hb][:],
                                             start=(kt == 0),
                                             stop=(kt == n_kt - 1))
                    # normalize both heads of the pair
                    cn_t = cn_p.tile([128, 512], dtr, tag="cn")
                    for hb, ctx_ps in ((0, ctx_e), (1, ctx_o)):
                        h = 2 * hp + hb
                        pb = hb * 64
                        r_row = r_p.tile([1, 512], dtr, tag="r")
                        nc.scalar.activation(r_row[:], ctx_ps[64:65, :], CPY)
                        bc_ps = ps.tile([128, 512], dtf, tag="ps")
                        nc.tensor.matmul(bc_ps[:], ones_t[:], r_row[:],
                                         start=True, stop=True)
                        rcp = rcp_p.tile([64, 512], dtf, tag="rcp")
                        nc.vector.reciprocal_approx_fast(rcp[:], bc_ps[0:64, :])
                        nc.vector.tensor_tensor(cn_t[pb:pb + 64, :],
                                                ctx_ps[0:64, :], rcp[:], MULT)
                        nc.vector.tensor_scalar_add(
                            cn_t[pb:pb + 64, :], cn_t[pb:pb + 64, :],
                            bv_sb[pb:pb + 64, hp:hp + 1])
                    cns.append(cn_t)

                # ---- out projection for this q-tile ----
                for mi in range(4):
                    for oc in range(2):
                        psy = ps.tile([128, 512], dtf, tag="ps")
                        for hp in range(4):
                            nc.tensor.matmul(
                                psy[:],
                                cns[hp][:, mi * 128:(mi + 1) * 128],
                                wout_sb[:, hp, oc, :],
                                start=(hp == 0), stop=(hp == 3))
                        y_sb = y_p.tile([128, 512], dtf, tag="y")
                        nc.vector.tensor_copy(y_sb[:], psy[:])
                        nc.sync.dma_start(
                            y_d[qt * 512 + mi * 128: qt * 512 + (mi + 1) * 128,
                                oc * 512:(oc + 1) * 512],
                            y_sb[:])
    nc.compile()
    return nc


def _host_shards(x, Wqkv, bqkv, Wout):
    import ml_dtypes
    mask = np.zeros((4, 128, 512), np.float32)
    qq = np.arange(512)[None, :]
    kk = np.arange(128)[:, None]
    for di in range(4):
        mask[di] = (kk + di * 128 <= qq)
    mask = mask.astype(ml_dtypes.bfloat16)

    in_maps = []
    for c in range(NCORES):
        b, hg = c // 2, c % 2
        s = hg * 512
        xt = np.ascontiguousarray(x[b].T)
        wqk = np.ascontiguousarray(
            np.concatenate([Wqkv[:, s:s + 512] * 0.125,
                            Wqkv[:, 1024 + s:1024 + s + 512]], axis=1))
        wv = np.ascontiguousarray(Wqkv[:, 2048 + s:2048 + s + 512])
        wout = np.ascontiguousarray(Wout[s:s + 512, :])
        bqk = np.concatenate([bqkv[s:s + 512] * 0.125,
                              bqkv[1024 + s:1024 + s + 512]]).astype(np.float32)
        bv = np.ascontiguousarray(bqkv[2048 + s:2048 + s + 512]).astype(np.float32)
        in_maps.append({"xt": xt, "wqk": wqk, "wv": wv, "wout": wout,
                       "mask": mask, "bqk": bqk, "bv": bv})
    return in_maps


_CACHED = {}


def kernel(x, Wqkv, bqkv, Wout, bout):
    from concourse.bass_utils import run_bass_kernel_spmd

    x = np.asarray(x, dtype=np.float32)
    Wqkv = np.asarray(Wqkv, dtype=np.float32)
    bqkv = np.asarray(bqkv, dtype=np.float32)
    Wout = np.asarray(Wout, dtype=np.float32)
    bout = np.asarray(bout, dtype=np.float32)
    assert x.shape == (B, T, C), x.shape

    if 'nc' not in _CACHED:
        _CACHED['nc'] = _build_program()
    nc = _CACHED['nc']

    in_maps = _host_shards(x, Wqkv, bqkv, Wout)
    res = run_bass_kernel_spmd(nc, in_maps, core_ids=list(range(NCORES)))

    y = np.empty((B, T, C), np.float32)
    for b in range(B):
        y[b] = res.results[2 * b]["y"] + res.results[2 * b + 1]["y"] + bout
    return y
